# revision 1
# baseline (speedup 1.0000x reference)
"""MultiHeadGAT layer on 8 Trainium2 NeuronCores — v2 (batched gathers).

Strategy (graph/data parallel, dst-sharded, per sharding hint):
  - Nodes partitioned into 8 ranges (6250/core); each core owns its output
    rows.  Edges routed host-side to the core/tile owning their destination,
    padded to 128-edge chunks.  All params + x replicated; no collectives.
  - Phase B (per core, replicated): node table in DRAM with 256-byte rows
        row(n) = [ xl(n) as fp8e4 (128B) | xl(n).att_dst as bf16 x4 | pad ]
    where xl = x @ W_lin^T.  The per-edge source-side attention dot thus
    rides the gather for free.  Also s_own[d,h] = xl(d).att_src for the
    core's own nodes (dst side).
  - Phase C per dst tile (128 nodes): ONE dma_gather per (supertile, src
    half) fetches all source rows (256B each).  int16 gather indices force
    a split into src<32768 / src>=32768 halves (separate tables).
  - alpha = lrelu(aj_gathered + OHdt^T s_own + eaT^T C16) via PSUM;
    e = exp(alpha) (no segment-max shift; alphas bounded); msg = e * xl;
    one-hot matmul accumulates [numerator | denom] per tile.  One-hot
    matrices (both orientations) are streamed from the host as fp8
    (index bookkeeping only — content is 0/1 at host-known positions).
  - Epilogue batched 8 tiles wide: divide, +bias, +residual, LayerNorm
    (rstd via exp(-0.5 ln(var+eps)) — keeps ACT on one table set), ELU.
"""

import math

import numpy as np

import concourse.bass as bass
import concourse.bacc as bacc
import concourse.mybir as mybir
from concourse.tile import TileContext
from concourse.masks import make_identity
from concourse.bass_utils import run_bass_kernel_spmd

F32 = mybir.dt.float32
BF16 = mybir.dt.bfloat16
FP8 = mybir.dt.float8e4
U8 = mybir.dt.uint8
I16 = mybir.dt.int16
AF = mybir.ActivationFunctionType
OP = mybir.AluOpType
AX = mybir.AxisListType

H, C = 4, 32
HC = H * C          # 128
IN_CH = 128
ED = 16
NEG_SLOPE = 0.2
LN_EPS = 1e-5
P = 128
ROW = 256           # table row bytes: 128 fp8 xl | 4 bf16 aj | 120 pad
SPLIT = 32768       # int16 index limit: src < SPLIT -> lo table
S_SUP = 2           # tiles per supertile (gather batch)
T_EP = 8            # tiles per epilogue batch

N_NODES = 50000
N_CORES = 8
NPC = N_NODES // N_CORES          # 6250
TILES = math.ceil(NPC / P)        # 49
NPAD = TILES * P                  # 6272
NT_TBL = math.ceil(N_NODES / P)   # 391
N_TBL = NT_TBL * P                # 50048
NT_LO = SPLIT // P                # 256 table tiles in lo table
N_HI = N_TBL - SPLIT              # 17280
NT_HI = NT_TBL - NT_LO            # 135

ONE_FP8 = 0x38  # 1.0 in float8_e4m3


def supertiles():
    out = []
    t = 0
    while t < TILES:
        s = min(S_SUP, TILES - t)
        out.append((t, s))
        t += s
    return out


# --------------------------------------------------------------------------
# host-side routing (index bookkeeping + layout only)
# --------------------------------------------------------------------------

def host_prep(x, edge_index, edge_attr, W_lin, W_edge, att_src, att_dst,
              att_edge, bias, ln_gamma, ln_beta):
    import ml_dtypes
    bf = ml_dtypes.bfloat16

    src = np.asarray(edge_index[0], np.int64)
    dst = np.asarray(edge_index[1], np.int64)
    ea = np.asarray(edge_attr, np.float32)
    E = src.shape[0]

    core_of = dst // NPC
    local = dst - core_of * NPC
    tile_of = local >> 7
    rel = local & 127
    half = (src >= SPLIT).astype(np.int64)

    key = (core_of * TILES + tile_of) * 2 + half
    order = np.argsort(key, kind="stable")
    key_s = key[order]
    counts = np.bincount(key_s, minlength=N_CORES * TILES * 2)
    cnt = counts.reshape(N_CORES, TILES, 2)
    M_LO = max(1, int(math.ceil(cnt[:, :, 0].max() / P)))
    M_HI = max(1, int(math.ceil(cnt[:, :, 1].max() / P)))
    M = M_LO + M_HI
    C_TOT = TILES * M

    group_start = np.zeros(N_CORES * TILES * 2, np.int64)
    np.cumsum(counts[:-1], out=group_start[1:])
    rank = np.arange(E, dtype=np.int64) - group_start[key_s]

    # chunk index in the per-core stream (supertile slot order):
    #   supertile st covers tiles [2st, 2st+s);  chunks
    #   [t0lo | t1lo | t0hi | t1hi] with global base st*2*M.
    t_s = tile_of[order]
    h_s = half[order]
    c_s = core_of[order]
    r_s = rank          # already in sorted order
    rel_s = rel[order]
    src_s = src[order]
    ea_s = ea[order]

    st = t_s // S_SUP
    pos = t_s - st * S_SUP
    s_t = np.minimum(S_SUP, TILES - st * S_SUP)  # tiles in this supertile
    base = st * S_SUP * M
    chunk = np.where(
        h_s == 0,
        base + pos * M_LO + (r_s >> 7),
        base + s_t * M_LO + pos * M_HI + (r_s >> 7),
    )
    lane = r_s & 127

    # lo/hi slot order (for gather indices): lo chunks tile-major
    lo_chunk = t_s * M_LO + (r_s >> 7)
    hi_chunk = t_s * M_HI + (r_s >> 7)

    x = np.asarray(x, np.float32)
    xT = np.zeros((IN_CH, N_TBL), np.float32)
    xT[:, :N_NODES] = x.T
    xT = np.ascontiguousarray(xT).astype(bf)

    wl = np.asarray(W_lin, np.float32).astype(bf)
    we = np.asarray(W_edge, np.float32).astype(bf)
    a_src = np.asarray(att_src, np.float32).reshape(HC, 1).astype(bf)
    a_dst = np.asarray(att_dst, np.float32).reshape(HC, 1).astype(bf)
    a_edge = np.asarray(att_edge, np.float32).reshape(HC, 1).astype(bf)
    bias_r = np.asarray(bias, np.float32).reshape(1, HC)
    gamma_r = np.asarray(ln_gamma, np.float32).reshape(1, HC)
    beta_r = np.asarray(ln_beta, np.float32).reshape(1, HC)

    def wrap16(a):
        # gather idx layout: idx j -> partition j%16, col j//16; replicate
        # to all 8 gpsimd core groups (128 partitions).
        cols = a.reshape(-1, 16).T  # [16, n/16]
        return np.ascontiguousarray(np.tile(cols, (8, 1)))

    in_maps = []
    for c in range(N_CORES):
        m = c_s == c
        ch = chunk[m]
        ln = lane[m]
        rl = rel_s[m]
        hh = h_s[m]
        sc = src_s[m]

        idx_lo = np.zeros(TILES * M_LO * P, np.int16)
        sel = ~hh.astype(bool)
        idx_lo[lo_chunk[m][sel] * P + ln[sel]] = sc[sel].astype(np.int16)
        idx_hi = np.zeros(TILES * M_HI * P, np.int16)
        selh = hh.astype(bool)
        idx_hi[hi_chunk[m][selh] * P + ln[selh]] = (
            sc[selh] - SPLIT).astype(np.int16)

        oh = np.zeros((C_TOT, P, P), np.uint8)
        oh[ch, ln, rl] = ONE_FP8
        ohdt = np.ascontiguousarray(oh.transpose(0, 2, 1))

        eat = np.zeros((C_TOT, P, ED), np.float32)
        eat[ch, ln] = ea_s[m]
        eat = eat.astype(bf)

        n0 = c * NPC
        xres = np.zeros((NPAD, IN_CH), np.float32)
        xres[:NPC] = x[n0:n0 + NPC]
        xresT = np.ascontiguousarray(xres.T).astype(bf)

        in_maps.append(dict(
            xT=xT,
            xresT=xresT,
            xres=xres,
            idx_lo=wrap16(idx_lo),
            idx_hi=wrap16(idx_hi),
            oh=oh,
            ohdt=ohdt,
            ea_sw=eat,
            wl=wl,
            we=we,
            a_src=a_src,
            a_dst=a_dst,
            a_edge=a_edge,
            bias=bias_r,
            ln_gamma=gamma_r,
            ln_beta=beta_r,
        ))
    return in_maps, M_LO, M_HI


# --------------------------------------------------------------------------
# device program
# --------------------------------------------------------------------------

def build_program(M_LO, M_HI, num_devices=None, debug_stage=5):
    M = M_LO + M_HI
    C_TOT = TILES * M
    PB = 3  # phase-B tiles per psum bank (3*132 <= 512)

    nc = bacc.Bacc("TRN2", target_bir_lowering=False, debug=False,
                   num_devices=num_devices or N_CORES)

    dp = nc.declare_dram_parameter
    xT_d = dp("xT", [IN_CH, N_TBL], BF16, isOutput=False)
    xresT_d = dp("xresT", [IN_CH, NPAD], BF16, isOutput=False)
    xres_d = dp("xres", [NPAD, IN_CH], F32, isOutput=False)
    idxlo_d = dp("idx_lo", [P, TILES * M_LO * 8], I16, isOutput=False)
    idxhi_d = dp("idx_hi", [P, TILES * M_HI * 8], I16, isOutput=False)
    oh_d = dp("oh", [C_TOT, P, P], U8, isOutput=False)
    ohdt_d = dp("ohdt", [C_TOT, P, P], U8, isOutput=False)
    ea_d = dp("ea_sw", [C_TOT, P, ED], BF16, isOutput=False)
    wl_d = dp("wl", [HC, IN_CH], BF16, isOutput=False)
    we_d = dp("we", [HC, ED], BF16, isOutput=False)
    asrc_d = dp("a_src", [HC, 1], BF16, isOutput=False)
    adst_d = dp("a_dst", [HC, 1], BF16, isOutput=False)
    aedge_d = dp("a_edge", [HC, 1], BF16, isOutput=False)
    bias_d = dp("bias", [1, HC], F32, isOutput=False)
    gamma_d = dp("ln_gamma", [1, HC], F32, isOutput=False)
    beta_d = dp("ln_beta", [1, HC], F32, isOutput=False)
    out_d = dp("out", [NPAD, HC], F32, isOutput=True)

    tbl_lo = nc.dram_tensor("tbl_lo", [SPLIT, ROW], U8)
    tbl_hi = nc.dram_tensor("tbl_hi", [N_HI, ROW], U8)

    with TileContext(nc) as tc:
        with (
            tc.tile_pool(name="const", bufs=1) as cpool,
            tc.tile_pool(name="pb", bufs=2) as bpool,
            tc.tile_pool(name="gath", bufs=2) as gpool,
            tc.tile_pool(name="stream", bufs=2) as spool,
            tc.tile_pool(name="work", bufs=2) as wpool,
            tc.tile_pool(name="ep", bufs=2) as epool,
            tc.tile_pool(name="ps_b", bufs=2, space="PSUM") as pbpool,
            tc.tile_pool(name="ps_al", bufs=2, space="PSUM") as papool,
            tc.tile_pool(name="ps_acc", bufs=3, space="PSUM") as pcpool,
        ):
            # ---------------- phase A: constants --------------------------
            ident = cpool.tile([P, P], BF16, tag="ident")
            make_identity(nc, ident[:])

            wl_sb = cpool.tile([HC, IN_CH], BF16, tag="wl")
            nc.sync.dma_start(out=wl_sb[:], in_=wl_d[:])
            we_sb = cpool.tile([HC, ED], BF16, tag="we")
            nc.sync.dma_start(out=we_sb[:], in_=we_d[:])
            asrc = cpool.tile([HC, 1], BF16, tag="asrc")
            nc.sync.dma_start(out=asrc[:], in_=asrc_d[:])
            adst = cpool.tile([HC, 1], BF16, tag="adst")
            nc.sync.dma_start(out=adst[:], in_=adst_d[:])
            aedge = cpool.tile([HC, 1], BF16, tag="aedge")
            nc.sync.dma_start(out=aedge[:], in_=aedge_d[:])

            # block-diagonal attention matrices [HC, 3H]:
            # cols 0:H att_dst (aj, src side), H:2H att_src (s_own, dst
            # side), 2H:3H att_edge.
            a_bd = cpool.tile([HC, 3 * H], BF16, tag="a_bd")
            nc.gpsimd.memset(a_bd[:], 0.0)
            for h in range(H):
                sl = slice(h * C, (h + 1) * C)
                nc.vector.tensor_copy(out=a_bd[sl, h:h + 1], in_=adst[sl, :])
                nc.vector.tensor_copy(out=a_bd[sl, H + h:H + h + 1],
                                      in_=asrc[sl, :])
                nc.vector.tensor_copy(out=a_bd[sl, 2 * H + h:2 * H + h + 1],
                                      in_=aedge[sl, :])

            # rhsBT [in_ch, 136] = [ W_lin^T | B_dst(aj) | B_src(s_own) ]
            rhsBT = cpool.tile([IN_CH, HC + 2 * H], BF16, tag="rhsbt")
            wlT_ps = pbpool.tile([P, P], BF16, tag="psA")
            nc.tensor.transpose(out=wlT_ps[:], in_=wl_sb[:], identity=ident[:])
            nc.scalar.copy(out=rhsBT[:, 0:HC], in_=wlT_ps[:])
            b8_ps = pbpool.tile([IN_CH, 2 * H], F32, tag="psA")
            nc.tensor.matmul(out=b8_ps[:], lhsT=wl_sb[:],
                             rhs=a_bd[:, 0:2 * H], start=True, stop=True)
            nc.vector.tensor_copy(out=rhsBT[:, HC:HC + 2 * H], in_=b8_ps[:])

            c16_ps = pbpool.tile([ED, H], F32, tag="psA")
            nc.tensor.matmul(out=c16_ps[:], lhsT=we_sb[:],
                             rhs=a_bd[:, 2 * H:3 * H], start=True, stop=True)
            c16 = cpool.tile([ED, H], BF16, tag="c16")
            nc.vector.tensor_copy(out=c16[:], in_=c16_ps[:])
            # c16T [H, ED] -> broadcast [P, H, ED] for the DVE ae product
            c16t_ps = pbpool.tile([H, ED], BF16, tag="psA")
            nc.tensor.transpose(out=c16t_ps[:], in_=c16[:],
                                identity=ident[0:ED, 0:ED])
            c16t = cpool.tile([H, ED], BF16, tag="c16t")
            nc.vector.tensor_copy(out=c16t[:], in_=c16t_ps[:])
            c16t_dram = nc.dram_tensor("c16t_scratch", [H, ED], BF16)
            nc.sync.dma_start(out=c16t_dram[:], in_=c16t[:])
            c16b = cpool.tile([P, H, ED], BF16, tag="c16b")
            nc.sync.dma_start(
                out=c16b[:],
                in_=c16t_dram[:].rearrange("a b -> (a b)")
                    .unsqueeze(0).to_broadcast([P, H * ED]))

            bias_b = cpool.tile([P, HC], F32, tag="bias_b")
            nc.sync.dma_start(out=bias_b[:], in_=bias_d[:].to_broadcast([P, HC]))
            gamma_b = cpool.tile([P, HC], F32, tag="gamma_b")
            nc.sync.dma_start(out=gamma_b[:],
                              in_=gamma_d[:].to_broadcast([P, HC]))
            beta_b = cpool.tile([P, HC], F32, tag="beta_b")
            nc.sync.dma_start(out=beta_b[:], in_=beta_d[:].to_broadcast([P, HC]))

            eps_t = cpool.tile([P, 1], F32, tag="eps_t")
            nc.gpsimd.memset(eps_t[:], LN_EPS)
            tiny_t = cpool.tile([P, 1], F32, tag="tiny_t")
            nc.gpsimd.memset(tiny_t[:], 1e-16)

            # s_own [128, TILES*H] bf16 (xl . att_src for own nodes)
            xresT_sb = cpool.tile([IN_CH, NPAD], BF16, tag="xresT")
            nc.sync.dma_start(out=xresT_sb[:], in_=xresT_d[:])
            s_own = cpool.tile([P, TILES * H], BF16, tag="s_own")
            for t in range(TILES):
                so_ps = pbpool.tile([P, H], F32, tag="psA")
                nc.tensor.matmul(out=so_ps[:],
                                 lhsT=xresT_sb[:, t * P:(t + 1) * P],
                                 rhs=rhsBT[:, HC + H:HC + 2 * H],
                                 start=True, stop=True)
                nc.vector.tensor_copy(out=s_own[:, t * H:(t + 1) * H],
                                      in_=so_ps[:])

            # ---------------- phase B: node table -------------------------
            # two persistent staging buffers (pad bytes memset once)
            stages = []
            for i in range(2):
                s = cpool.tile([P, PB, ROW], U8, name=f"stage{i}",
                               tag=f"stage{i}")
                nc.gpsimd.memset(s[:, :, HC + 2 * H:], 0)
                stages.append(s)
            n_b = math.ceil(NT_TBL / PB)
            for b in range(n_b):
                t0 = b * PB
                nt = min(PB, NT_TBL - t0)
                xt = bpool.tile([IN_CH, PB * P], BF16, tag="xt")
                nc.sync.dma_start(out=xt[:, :nt * P],
                                  in_=xT_d[:, t0 * P:(t0 + nt) * P])
                row_ps = pbpool.tile([P, PB, HC + H], F32, tag="psA")
                for i in range(nt):
                    nc.tensor.matmul(out=row_ps[:, i, :],
                                     lhsT=xt[:, i * P:(i + 1) * P],
                                     rhs=rhsBT[:, 0:HC + H],
                                     start=True, stop=True)
                stage = stages[b % 2]
                nc.scalar.copy(out=stage[:, :nt, 0:HC].bitcast(FP8),
                               in_=row_ps[:, :nt, 0:HC])
                nc.vector.tensor_copy(
                    out=stage[:, :nt, HC:HC + 2 * H].bitcast(BF16),
                    in_=row_ps[:, :nt, HC:HC + H])
                if t0 + nt <= NT_LO:
                    dst = tbl_lo[t0 * P:(t0 + nt) * P, :]
                elif t0 >= NT_LO:
                    dst = tbl_hi[(t0 - NT_LO) * P:(t0 - NT_LO + nt) * P, :]
                else:
                    dst = None
                if dst is not None:
                    nc.scalar.dma_start(
                        out=dst.rearrange("(b p) r -> p b r", p=P),
                        in_=stage[:, :nt, :])
                else:
                    k = NT_LO - t0
                    nc.scalar.dma_start(
                        out=tbl_lo[t0 * P:SPLIT, :]
                            .rearrange("(b p) r -> p b r", p=P),
                        in_=stage[:, :k, :])
                    nc.scalar.dma_start(
                        out=tbl_hi[0:(nt - k) * P, :]
                            .rearrange("(b p) r -> p b r", p=P),
                        in_=stage[:, k:nt, :])

            # ---------------- phase C: edges ------------------------------
            stage_ep = None
            ep_fill = 0
            ep_base = 0

            def flush_epilogue(stage_ep, n_tiles, t0):
                # stage_ep: [P, T_EP, HC+H] f32, tiles t0..t0+n_tiles-1
                nt = n_tiles
                num = stage_ep[:, :nt, 0:HC]
                den = stage_ep[:, :nt, HC:HC + H]
                rden = epool.tile([P, T_EP, H], F32, tag="rden")
                nc.scalar.activation(out=rden[:, :nt, :], in_=den,
                                     func=AF.Identity, bias=tiny_t[:, 0:1])
                nc.vector.reciprocal(out=rden[:, :nt, :], in_=rden[:, :nt, :])
                o = epool.tile([P, T_EP, HC], F32, tag="o")
                nc.vector.tensor_tensor(
                    out=o[:, :nt, :].rearrange("p t (h c) -> p t h c", c=C),
                    in0=num.rearrange("p t (h c) -> p t h c", c=C),
                    in1=rden[:, :nt, :].unsqueeze(3)
                        .to_broadcast([P, nt, H, C]),
                    op=OP.mult)
                nc.vector.tensor_tensor(
                    out=o[:, :nt, :], in0=o[:, :nt, :],
                    in1=bias_b[:].unsqueeze(1).to_broadcast([P, nt, HC]),
                    op=OP.add)
                xr = epool.tile([P, T_EP, HC], F32, tag="xr")
                nc.sync.dma_start(
                    out=xr[:, :nt, :],
                    in_=xres_d[t0 * P:(t0 + nt) * P, :]
                        .rearrange("(t p) c -> p t c", p=P))
                nc.vector.tensor_tensor(out=o[:, :nt, :], in0=o[:, :nt, :],
                                        in1=xr[:, :nt, :], op=OP.add)
                # LayerNorm across channels
                mu = epool.tile([P, T_EP], F32, tag="mu")
                nc.vector.reduce_sum(out=mu[:, :nt], in_=o[:, :nt, :],
                                     axis=AX.X)
                nc.scalar.mul(out=mu[:, :nt], in_=mu[:, :nt], mul=1.0 / HC)
                nc.vector.tensor_tensor(
                    out=o[:, :nt, :], in0=o[:, :nt, :],
                    in1=mu[:, :nt].unsqueeze(2).to_broadcast([P, nt, HC]),
                    op=OP.subtract)
                sq = epool.tile([P, T_EP, HC], F32, tag="sq")
                nc.vector.tensor_tensor(out=sq[:, :nt, :], in0=o[:, :nt, :],
                                        in1=o[:, :nt, :], op=OP.mult)
                var = epool.tile([P, T_EP], F32, tag="var")
                nc.vector.reduce_sum(out=var[:, :nt], in_=sq[:, :nt, :],
                                     axis=AX.X)
                # rstd = exp(-0.5 * ln(var/HC + eps))
                nc.scalar.activation(out=var[:, :nt], in_=var[:, :nt],
                                     func=AF.Ln, scale=1.0 / HC,
                                     bias=eps_t[:, 0:1])
                nc.scalar.activation(out=var[:, :nt], in_=var[:, :nt],
                                     func=AF.Exp, scale=-0.5)
                nc.vector.tensor_tensor(
                    out=o[:, :nt, :], in0=o[:, :nt, :],
                    in1=var[:, :nt].unsqueeze(2).to_broadcast([P, nt, HC]),
                    op=OP.mult)
                nc.vector.tensor_tensor(
                    out=o[:, :nt, :], in0=o[:, :nt, :],
                    in1=gamma_b[:].unsqueeze(1).to_broadcast([P, nt, HC]),
                    op=OP.mult)
                nc.vector.tensor_tensor(
                    out=o[:, :nt, :], in0=o[:, :nt, :],
                    in1=beta_b[:].unsqueeze(1).to_broadcast([P, nt, HC]),
                    op=OP.add)
                # ELU = relu(x) + min(exp(x)-1, 0)
                ex = epool.tile([P, T_EP, HC], F32, tag="ex")
                nc.scalar.activation(out=ex[:, :nt, :], in_=o[:, :nt, :],
                                     func=AF.Exp)
                nc.vector.tensor_scalar(out=ex[:, :nt, :], in0=ex[:, :nt, :],
                                        scalar1=-1.0, scalar2=0.0,
                                        op0=OP.add, op1=OP.min)
                nc.scalar.activation(out=o[:, :nt, :], in_=o[:, :nt, :],
                                     func=AF.Relu)
                nc.vector.tensor_tensor(out=o[:, :nt, :], in0=o[:, :nt, :],
                                        in1=ex[:, :nt, :], op=OP.add)
                nc.sync.dma_start(
                    out=out_d[t0 * P:(t0 + nt) * P, :]
                        .rearrange("(t p) c -> p t c", p=P),
                    in_=o[:, :nt, :])

            if debug_stage < 4:
                zout = cpool.tile([P, TILES, HC], F32, tag="zout")
                nc.gpsimd.memset(zout[:], 0.0)
                nc.sync.dma_start(
                    out=out_d[:].rearrange("(t p) c -> p t c", p=P),
                    in_=zout[:])

            for (tb, s_t) in supertiles():
                if debug_stage < 2:
                    break
                nch = s_t * M                   # chunks this supertile
                cb = tb * M                     # global chunk base
                n_lo = s_t * M_LO
                n_hi = s_t * M_HI

                idx_lo = gpool.tile([P, S_SUP * M_LO * 8], I16, tag="idxlo")
                nc.sync.dma_start(
                    out=idx_lo[:, :n_lo * 8],
                    in_=idxlo_d[:, tb * M_LO * 8:(tb * M_LO + n_lo) * 8])
                idx_hi = gpool.tile([P, S_SUP * M_HI * 8], I16, tag="idxhi")
                nc.sync.dma_start(
                    out=idx_hi[:, :n_hi * 8],
                    in_=idxhi_d[:, tb * M_HI * 8:(tb * M_HI + n_hi) * 8])

                g = gpool.tile([P, S_SUP * M, ROW], U8, tag="g")
                GB = 8  # chunks per gather (1024 idx limit)
                for c0 in range(0, n_lo, GB):
                    k = min(GB, n_lo - c0)
                    nc.gpsimd.dma_gather(
                        g[:, c0:c0 + k, :], tbl_lo[:],
                        idx_lo[:, c0 * 8:(c0 + k) * 8],
                        k * P, k * P, ROW)
                for c0 in range(0, n_hi, GB):
                    k = min(GB, n_hi - c0)
                    nc.gpsimd.dma_gather(
                        g[:, n_lo + c0:n_lo + c0 + k, :], tbl_hi[:],
                        idx_hi[:, c0 * 8:(c0 + k) * 8],
                        k * P, k * P, ROW)

                if debug_stage < 3:
                    continue
                oh_sb = spool.tile([P, S_SUP * M, P], U8, tag="oh")
                nc.sync.dma_start(
                    out=oh_sb[:, :nch, :],
                    in_=oh_d[cb:cb + nch].rearrange("c p d -> p c d"))
                ohdt_sb = spool.tile([P, S_SUP * M, P], U8, tag="ohdt")
                nc.sync.dma_start(
                    out=ohdt_sb[:, :nch, :],
                    in_=ohdt_d[cb:cb + nch].rearrange("c d e -> d c e"))
                ea_sb = spool.tile([P, S_SUP * M, ED], BF16, tag="ea")
                nc.sync.dma_start(
                    out=ea_sb[:, :nch, :],
                    in_=ea_d[cb:cb + nch].rearrange("c p e -> p c e"))

                # chunk -> (tile, is_first, is_last) in supertile slot order
                owner = ([tb + p for p in range(s_t) for _ in range(M_LO)]
                         + [tb + p for p in range(s_t) for _ in range(M_HI)])
                first = {tb + p: p * M_LO for p in range(s_t)}
                last = {tb + p: n_lo + p * M_HI + M_HI - 1
                        for p in range(s_t)}

                # sdst = OHdt^T s_own   (per chunk, 4 cols)
                al_ps = papool.tile([P, S_SUP * M * H], F32, tag="al")
                for cidx in range(nch):
                    t = owner[cidx]
                    sl = slice(cidx * H, (cidx + 1) * H)
                    nc.tensor.matmul(
                        out=al_ps[:, sl],
                        lhsT=ohdt_sb[:, cidx, :].bitcast(FP8),
                        rhs=s_own[:, t * H:(t + 1) * H],
                        start=True, stop=True, skip_group_check=True)

                # ae = (ea * C16t) tree-summed over ED, on DVE (bf16)
                prod = wpool.tile([P, S_SUP * M, H, ED], BF16, tag="prod")
                nc.vector.tensor_tensor(
                    out=prod[:, :nch],
                    in0=ea_sb[:, :nch, :].unsqueeze(2)
                        .to_broadcast([P, nch, H, ED]),
                    in1=c16b[:].unsqueeze(1).to_broadcast([P, nch, H, ED]),
                    op=OP.mult)
                for w in (8, 4, 2, 1):
                    nc.vector.tensor_tensor(
                        out=prod[:, :nch, :, 0:w],
                        in0=prod[:, :nch, :, 0:w],
                        in1=prod[:, :nch, :, w:2 * w],
                        op=OP.add)

                # alpha += aj (gathered);  lrelu;  exp -> msg tail
                alpha = wpool.tile([P, S_SUP * M, H], F32, tag="alpha")
                nc.vector.tensor_tensor(
                    out=alpha[:, :nch, :],
                    in0=al_ps[:, :nch * H].rearrange("p (c h) -> p c h", h=H),
                    in1=g[:, :nch, HC:HC + 2 * H].bitcast(BF16),
                    op=OP.add)
                nc.vector.tensor_tensor(
                    out=alpha[:, :nch, :],
                    in0=alpha[:, :nch, :],
                    in1=prod[:, :nch, :, 0],
                    op=OP.add)
                nc.vector.scalar_tensor_tensor(
                    out=alpha[:, :nch, :], in0=alpha[:, :nch, :],
                    scalar=NEG_SLOPE, in1=alpha[:, :nch, :],
                    op0=OP.mult, op1=OP.max)
                msg = wpool.tile([P, S_SUP * M, HC + H], BF16, tag="msg")
                nc.scalar.activation(out=msg[:, :nch, HC:HC + H],
                                     in_=alpha[:, :nch, :], func=AF.Exp)
                # msg = e * xl
                nc.vector.tensor_tensor(
                    out=msg[:, :nch, 0:HC].rearrange(
                        "p c (h w) -> p c h w", w=C),
                    in0=g[:, :nch, 0:HC].bitcast(FP8).rearrange(
                        "p c (h w) -> p c h w", w=C),
                    in1=msg[:, :nch, HC:HC + H].unsqueeze(3)
                        .to_broadcast([P, nch, H, C]),
                    op=OP.mult)

                if debug_stage < 4:
                    continue
                # accumulate [numerator | denom] per tile
                accs = {}
                for cidx in range(nch):
                    t = owner[cidx]
                    if cidx == first[t]:
                        accs[t] = pcpool.tile([P, HC + H], F32,
                                              name="acc_t", tag="acc")
                    nc.tensor.matmul(
                        out=accs[t][:],
                        lhsT=oh_sb[:, cidx, :].bitcast(FP8),
                        rhs=msg[:, cidx, :],
                        start=(cidx == first[t]), stop=(cidx == last[t]))

                for p in range(s_t):
                    t = tb + p
                    if ep_fill == 0:
                        stage_ep = epool.tile([P, T_EP, HC + H], F32,
                                              tag="stage_ep")
                        ep_base = t
                    nc.vector.tensor_copy(out=stage_ep[:, t - ep_base, :],
                                          in_=accs[t][:])
                    ep_fill += 1
                    if ep_fill == T_EP or t == TILES - 1:
                        if debug_stage >= 5:
                            flush_epilogue(stage_ep, ep_fill, ep_base)
                        else:
                            nc.sync.dma_start(
                                out=out_d[ep_base * P:(ep_base + ep_fill) * P, :]
                                    .rearrange("(t p) c -> p t c", p=P),
                                in_=stage_ep[:, :ep_fill, 0:HC])
                        ep_fill = 0

    nc.compile()
    return nc


# --------------------------------------------------------------------------
# entry point
# --------------------------------------------------------------------------

def kernel(**inputs) -> np.ndarray:
    in_maps, M_LO, M_HI = host_prep(**inputs)
    nc = build_program(M_LO, M_HI)
    res = run_bass_kernel_spmd(nc, in_maps, list(range(N_CORES)))
    parts = [res.results[c]["out"][:NPC] for c in range(N_CORES)]
    return np.concatenate(parts, axis=0).astype(np.float32)



# revision 11
# speedup vs baseline: 2.2474x; 2.2474x over previous
"""MultiHeadGAT layer on 8 Trainium2 NeuronCores — v2 (batched gathers).

Strategy (graph/data parallel, dst-sharded, per sharding hint):
  - Nodes partitioned into 8 ranges (6250/core); each core owns its output
    rows.  Edges routed host-side to the core/tile owning their destination,
    padded to 128-edge chunks.  All params + x replicated; no collectives.
  - Phase B (per core, replicated): node table in DRAM with 256-byte rows
        row(n) = [ xl(n) as fp8e4 (128B) | xl(n).att_dst as bf16 x4 | pad ]
    where xl = x @ W_lin^T.  The per-edge source-side attention dot thus
    rides the gather for free.  Also s_own[d,h] = xl(d).att_src for the
    core's own nodes (dst side).
  - Phase C per dst tile (128 nodes): ONE dma_gather per (supertile, src
    half) fetches all source rows (256B each).  int16 gather indices force
    a split into src<32768 / src>=32768 halves (separate tables).
  - alpha = lrelu(aj_gathered + OHdt^T s_own + eaT^T C16) via PSUM;
    e = exp(alpha) (no segment-max shift; alphas bounded); msg = e * xl;
    one-hot matmul accumulates [numerator | denom] per tile.  One-hot
    matrices (both orientations) are streamed from the host as fp8
    (index bookkeeping only — content is 0/1 at host-known positions).
  - Epilogue batched 8 tiles wide: divide, +bias, +residual, LayerNorm
    (rstd via exp(-0.5 ln(var+eps)) — keeps ACT on one table set), ELU.
"""

import math

import numpy as np

import concourse.bass as bass
import concourse.bacc as bacc
import concourse.mybir as mybir
from concourse.tile import TileContext
from concourse.masks import make_identity
from concourse.bass_utils import run_bass_kernel_spmd

F32 = mybir.dt.float32
BF16 = mybir.dt.bfloat16
FP8 = mybir.dt.float8e4
U8 = mybir.dt.uint8
I16 = mybir.dt.int16
AF = mybir.ActivationFunctionType
OP = mybir.AluOpType
AX = mybir.AxisListType

H, C = 4, 32
HC = H * C          # 128
IN_CH = 128
ED = 16
NEG_SLOPE = 0.2
LN_EPS = 1e-5
P = 128
ROW = 256           # table row bytes: 128 fp8 xl | 4 bf16 aj | 120 pad
SPLIT = 32768       # int16 index limit: src < SPLIT -> lo table
S_SUP = 2           # tiles per supertile (gather batch)
T_EP = 8            # tiles per epilogue batch

N_NODES = 50000
N_CORES = 8
NPC = N_NODES // N_CORES          # 6250
TILES = math.ceil(NPC / P)        # 49
NPAD = TILES * P                  # 6272
NT_TBL = math.ceil(N_NODES / P)   # 391
N_TBL = NT_TBL * P                # 50048
NT_LO = SPLIT // P                # 256 table tiles in lo table
N_HI = N_TBL - SPLIT              # 17280
NT_HI = NT_TBL - NT_LO            # 135

ONE_FP8 = 0x38  # 1.0 in float8_e4m3


def supertiles():
    out = []
    t = 0
    while t < TILES:
        s = min(S_SUP, TILES - t)
        out.append((t, s))
        t += s
    return out


# --------------------------------------------------------------------------
# host-side routing (index bookkeeping + layout only)
# --------------------------------------------------------------------------

def host_prep(x, edge_index, edge_attr, W_lin, W_edge, att_src, att_dst,
              att_edge, bias, ln_gamma, ln_beta):
    import ml_dtypes
    bf = ml_dtypes.bfloat16

    src = np.asarray(edge_index[0], np.int64)
    dst = np.asarray(edge_index[1], np.int64)
    ea = np.asarray(edge_attr, np.float32)
    E = src.shape[0]

    core_of = dst // NPC
    local = dst - core_of * NPC
    tile_of = local >> 7
    rel = local & 127
    # table rows are partition-major: node n = b*128+p lives at row
    # pi(n) = p*NT_TBL + b, so phase-B writes are contiguous per partition.
    pi_src = (src & 127) * NT_TBL + (src >> 7)
    half = (pi_src >= SPLIT).astype(np.int64)

    key = (core_of * TILES + tile_of) * 2 + half
    order = np.argsort(key, kind="stable")
    key_s = key[order]
    counts = np.bincount(key_s, minlength=N_CORES * TILES * 2)
    cnt = counts.reshape(N_CORES, TILES, 2)
    M_LO = max(1, int(math.ceil(cnt[:, :, 0].max() / P)))
    M_HI = max(1, int(math.ceil(cnt[:, :, 1].max() / P)))
    M = M_LO + M_HI
    C_TOT = TILES * M

    group_start = np.zeros(N_CORES * TILES * 2, np.int64)
    np.cumsum(counts[:-1], out=group_start[1:])
    rank = np.arange(E, dtype=np.int64) - group_start[key_s]

    # chunk index in the per-core stream (supertile slot order):
    #   supertile st covers tiles [2st, 2st+s);  chunks
    #   [t0lo | t1lo | t0hi | t1hi] with global base st*2*M.
    t_s = tile_of[order]
    h_s = half[order]
    c_s = core_of[order]
    r_s = rank          # already in sorted order
    rel_s = rel[order]
    src_s = pi_src[order]   # table-row index (pi-permuted)
    ea_s = ea[order]

    st = t_s // S_SUP
    pos = t_s - st * S_SUP
    s_t = np.minimum(S_SUP, TILES - st * S_SUP)  # tiles in this supertile
    base = st * S_SUP * M
    chunk = np.where(
        h_s == 0,
        base + pos * M_LO + (r_s >> 7),
        base + s_t * M_LO + pos * M_HI + (r_s >> 7),
    )
    lane = r_s & 127

    # lo/hi slot order (for gather indices): lo chunks tile-major
    lo_chunk = t_s * M_LO + (r_s >> 7)
    hi_chunk = t_s * M_HI + (r_s >> 7)

    x = np.asarray(x, np.float32)
    xT = np.zeros((IN_CH, N_TBL), np.float32)
    xT[:, :N_NODES] = x.T
    xT = np.ascontiguousarray(xT).astype(bf)

    wl = np.asarray(W_lin, np.float32).astype(bf)
    we = np.asarray(W_edge, np.float32).astype(bf)
    a_src = np.asarray(att_src, np.float32).reshape(HC, 1).astype(bf)
    a_dst = np.asarray(att_dst, np.float32).reshape(HC, 1).astype(bf)
    a_edge = np.asarray(att_edge, np.float32).reshape(HC, 1).astype(bf)
    bias_r = np.asarray(bias, np.float32).reshape(1, HC)
    gamma_r = np.asarray(ln_gamma, np.float32).reshape(1, HC)
    beta_r = np.asarray(ln_beta, np.float32).reshape(1, HC)

    def wrap16(a):
        # gather idx layout: idx j -> partition j%16, col j//16; replicate
        # to all 8 gpsimd core groups (128 partitions).
        cols = a.reshape(-1, 16).T  # [16, n/16]
        return np.ascontiguousarray(np.tile(cols, (8, 1)))

    in_maps = []
    for c in range(N_CORES):
        m = c_s == c
        ch = chunk[m]
        ln = lane[m]
        rl = rel_s[m]
        hh = h_s[m]
        sc = src_s[m]

        idx_lo = np.zeros(TILES * M_LO * P, np.int16)
        sel = ~hh.astype(bool)
        idx_lo[lo_chunk[m][sel] * P + ln[sel]] = sc[sel].astype(np.int16)
        idx_hi = np.zeros(TILES * M_HI * P, np.int16)
        selh = hh.astype(bool)
        idx_hi[hi_chunk[m][selh] * P + ln[selh]] = (
            sc[selh] - SPLIT).astype(np.int16)

        # partition-major streams: [P, C_TOT, *] so the per-supertile DMA is
        # one contiguous descriptor per partition.
        oh = np.zeros((C_TOT, P, P), np.uint8)
        oh[ch, ln, rl] = ONE_FP8
        oh_pm = np.ascontiguousarray(oh.transpose(1, 0, 2))
        ohdt = np.ascontiguousarray(oh.transpose(2, 0, 1))

        eat = np.zeros((C_TOT, P, ED), np.float32)
        eat[ch, ln] = ea_s[m]
        eat = np.ascontiguousarray(eat.transpose(1, 0, 2)).astype(bf)

        n0 = c * NPC
        xres = np.zeros((NPAD, IN_CH), np.float32)
        xres[:NPC] = x[n0:n0 + NPC]
        xresT = np.ascontiguousarray(xres.T).astype(bf)

        in_maps.append(dict(
            xT=xT,
            xresT=xresT,
            xres=xres,
            idx_lo=wrap16(idx_lo),
            idx_hi=wrap16(idx_hi),
            oh=oh_pm,
            ohdt=ohdt,
            ea_sw=eat,
            wl=wl,
            we=we,
            a_src=a_src,
            a_dst=a_dst,
            a_edge=a_edge,
            bias=bias_r,
            ln_gamma=gamma_r,
            ln_beta=beta_r,
        ))
    return in_maps, M_LO, M_HI


# --------------------------------------------------------------------------
# device program
# --------------------------------------------------------------------------

def build_program(M_LO, M_HI, num_devices=None, debug_stage=5):
    M = M_LO + M_HI
    C_TOT = TILES * M
    PB = 3  # phase-B tiles per psum bank (3*132 <= 512)

    nc = bacc.Bacc("TRN2", target_bir_lowering=False, debug=False,
                   num_devices=num_devices or N_CORES,
                   num_swdge_queues=4)

    dp = nc.declare_dram_parameter
    xT_d = dp("xT", [IN_CH, N_TBL], BF16, isOutput=False)
    xresT_d = dp("xresT", [IN_CH, NPAD], BF16, isOutput=False)
    xres_d = dp("xres", [NPAD, IN_CH], F32, isOutput=False)
    idxlo_d = dp("idx_lo", [P, TILES * M_LO * 8], I16, isOutput=False)
    idxhi_d = dp("idx_hi", [P, TILES * M_HI * 8], I16, isOutput=False)
    oh_d = dp("oh", [P, C_TOT, P], U8, isOutput=False)
    ohdt_d = dp("ohdt", [P, C_TOT, P], U8, isOutput=False)
    ea_d = dp("ea_sw", [P, C_TOT, ED], BF16, isOutput=False)
    wl_d = dp("wl", [HC, IN_CH], BF16, isOutput=False)
    we_d = dp("we", [HC, ED], BF16, isOutput=False)
    asrc_d = dp("a_src", [HC, 1], BF16, isOutput=False)
    adst_d = dp("a_dst", [HC, 1], BF16, isOutput=False)
    aedge_d = dp("a_edge", [HC, 1], BF16, isOutput=False)
    bias_d = dp("bias", [1, HC], F32, isOutput=False)
    gamma_d = dp("ln_gamma", [1, HC], F32, isOutput=False)
    beta_d = dp("ln_beta", [1, HC], F32, isOutput=False)
    out_d = dp("out", [NPAD, HC], F32, isOutput=True)

    # pi-permuted node table: node n = b*128+p at row p*NT_TBL + b, so
    # phase-B writes land contiguously per partition.
    tbl = nc.dram_tensor("tbl", [N_TBL, ROW], U8)

    with TileContext(nc) as tc:
        with (
            tc.tile_pool(name="const", bufs=1) as cpool,
            tc.tile_pool(name="pb", bufs=2) as bpool,
            tc.tile_pool(name="gath", bufs=2) as gpool,
            tc.tile_pool(name="stream", bufs=2) as spool,
            tc.tile_pool(name="work", bufs=2) as wpool,
            tc.tile_pool(name="ep", bufs=2) as epool,
            tc.tile_pool(name="ps_b", bufs=2, space="PSUM") as pbpool,
            tc.tile_pool(name="ps_al", bufs=2, space="PSUM") as papool,
            tc.tile_pool(name="ps_acc", bufs=3, space="PSUM") as pcpool,
        ):
            # ---------------- phase A: constants --------------------------
            ident = cpool.tile([P, P], BF16, tag="ident")
            make_identity(nc, ident[:])

            wl_sb = cpool.tile([HC, IN_CH], BF16, tag="wl")
            nc.sync.dma_start(out=wl_sb[:], in_=wl_d[:])
            we_sb = cpool.tile([HC, ED], BF16, tag="we")
            nc.sync.dma_start(out=we_sb[:], in_=we_d[:])
            asrc = cpool.tile([HC, 1], BF16, tag="asrc")
            nc.sync.dma_start(out=asrc[:], in_=asrc_d[:])
            adst = cpool.tile([HC, 1], BF16, tag="adst")
            nc.sync.dma_start(out=adst[:], in_=adst_d[:])
            aedge = cpool.tile([HC, 1], BF16, tag="aedge")
            nc.sync.dma_start(out=aedge[:], in_=aedge_d[:])

            # block-diagonal attention matrices [HC, 3H]:
            # cols 0:H att_dst (aj, src side), H:2H att_src (s_own, dst
            # side), 2H:3H att_edge.
            a_bd = cpool.tile([HC, 3 * H], BF16, tag="a_bd")
            nc.gpsimd.memset(a_bd[:], 0.0)
            for h in range(H):
                sl = slice(h * C, (h + 1) * C)
                nc.vector.tensor_copy(out=a_bd[sl, h:h + 1], in_=adst[sl, :])
                nc.vector.tensor_copy(out=a_bd[sl, H + h:H + h + 1],
                                      in_=asrc[sl, :])
                nc.vector.tensor_copy(out=a_bd[sl, 2 * H + h:2 * H + h + 1],
                                      in_=aedge[sl, :])

            # rhsBT [in_ch, 136] = [ W_lin^T | B_dst(aj) | B_src(s_own) ]
            rhsBT = cpool.tile([IN_CH, HC + 2 * H], BF16, tag="rhsbt")
            wlT_ps = pbpool.tile([P, P], BF16, tag="psA")
            nc.tensor.transpose(out=wlT_ps[:], in_=wl_sb[:], identity=ident[:])
            nc.scalar.copy(out=rhsBT[:, 0:HC], in_=wlT_ps[:])
            b8_ps = pbpool.tile([IN_CH, 2 * H], F32, tag="psA")
            nc.tensor.matmul(out=b8_ps[:], lhsT=wl_sb[:],
                             rhs=a_bd[:, 0:2 * H], start=True, stop=True)
            nc.vector.tensor_copy(out=rhsBT[:, HC:HC + 2 * H], in_=b8_ps[:])

            c16_ps = pbpool.tile([ED, H], F32, tag="psA")
            nc.tensor.matmul(out=c16_ps[:], lhsT=we_sb[:],
                             rhs=a_bd[:, 2 * H:3 * H], start=True, stop=True)
            c16 = cpool.tile([ED, H], BF16, tag="c16")
            nc.vector.tensor_copy(out=c16[:], in_=c16_ps[:])
            # c16T [H, ED] -> broadcast [P, H, ED] for the DVE ae product
            c16t_ps = pbpool.tile([H, ED], BF16, tag="psA")
            nc.tensor.transpose(out=c16t_ps[:], in_=c16[:],
                                identity=ident[0:ED, 0:ED])
            c16t = cpool.tile([H, ED], BF16, tag="c16t")
            nc.vector.tensor_copy(out=c16t[:], in_=c16t_ps[:])
            c16t_dram = nc.dram_tensor("c16t_scratch", [H, ED], BF16)
            nc.sync.dma_start(out=c16t_dram[:], in_=c16t[:])
            c16b = cpool.tile([P, H, ED], BF16, tag="c16b")
            nc.sync.dma_start(
                out=c16b[:],
                in_=c16t_dram[:].rearrange("a b -> (a b)")
                    .unsqueeze(0).to_broadcast([P, H * ED]))

            bias_b = cpool.tile([P, HC], F32, tag="bias_b")
            nc.sync.dma_start(out=bias_b[:], in_=bias_d[:].to_broadcast([P, HC]))
            gamma_b = cpool.tile([P, HC], F32, tag="gamma_b")
            nc.sync.dma_start(out=gamma_b[:],
                              in_=gamma_d[:].to_broadcast([P, HC]))
            beta_b = cpool.tile([P, HC], F32, tag="beta_b")
            nc.sync.dma_start(out=beta_b[:], in_=beta_d[:].to_broadcast([P, HC]))

            eps_t = cpool.tile([P, 1], F32, tag="eps_t")
            nc.gpsimd.memset(eps_t[:], LN_EPS)
            tiny_t = cpool.tile([P, 1], F32, tag="tiny_t")
            nc.gpsimd.memset(tiny_t[:], 1e-16)

            # s_own [128, TILES*H] bf16 (xl . att_src for own nodes)
            xresT_sb = cpool.tile([IN_CH, NPAD], BF16, tag="xresT")
            nc.sync.dma_start(out=xresT_sb[:], in_=xresT_d[:])
            s_own = cpool.tile([P, TILES * H], BF16, tag="s_own")
            for t in range(TILES):
                so_ps = pbpool.tile([P, H], F32, tag="psA")
                nc.tensor.matmul(out=so_ps[:],
                                 lhsT=xresT_sb[:, t * P:(t + 1) * P],
                                 rhs=rhsBT[:, HC + H:HC + 2 * H],
                                 start=True, stop=True)
                nc.vector.tensor_copy(out=s_own[:, t * H:(t + 1) * H],
                                      in_=so_ps[:])

            # ---------------- phase B: node table -------------------------
            # two persistent staging buffers (pad bytes memset once)
            stages = []
            for i in range(2):
                s = cpool.tile([P, PB, ROW], U8, name=f"stage{i}",
                               tag=f"stage{i}")
                nc.gpsimd.memset(s[:, :, HC + 2 * H:], 0)
                stages.append(s)
            n_b = math.ceil(NT_TBL / PB)
            for b in range(n_b):
                t0 = b * PB
                nt = min(PB, NT_TBL - t0)
                xt = bpool.tile([IN_CH, PB * P], BF16, tag="xt")
                nc.sync.dma_start(out=xt[:, :nt * P],
                                  in_=xT_d[:, t0 * P:(t0 + nt) * P])
                row_ps = pbpool.tile([P, PB, HC + H], F32, tag="psA")
                for i in range(nt):
                    nc.tensor.matmul(out=row_ps[:, i, :],
                                     lhsT=xt[:, i * P:(i + 1) * P],
                                     rhs=rhsBT[:, 0:HC + H],
                                     start=True, stop=True)
                stage = stages[b % 2]
                nc.scalar.copy(out=stage[:, :nt, 0:HC].bitcast(FP8),
                               in_=row_ps[:, :nt, 0:HC])
                nc.vector.tensor_copy(
                    out=stage[:, :nt, HC:HC + 2 * H].bitcast(BF16),
                    in_=row_ps[:, :nt, HC:HC + H])
                nc.scalar.dma_start(
                    out=tbl[:].rearrange("(p b) r -> p b r", p=P)
                        [:, t0:t0 + nt, :],
                    in_=stage[:, :nt, :])

            # ---------------- phase C: edges ------------------------------
            stage_ep = None
            ep_fill = 0
            ep_base = 0

            def flush_epilogue(stage_ep, n_tiles, t0):
                # stage_ep: [P, T_EP, HC+H] f32, tiles t0..t0+n_tiles-1
                nt = n_tiles
                num = stage_ep[:, :nt, 0:HC]
                den = stage_ep[:, :nt, HC:HC + H]
                rden = epool.tile([P, T_EP, H], F32, tag="rden")
                nc.scalar.activation(out=rden[:, :nt, :], in_=den,
                                     func=AF.Identity, bias=tiny_t[:, 0:1])
                nc.vector.reciprocal(out=rden[:, :nt, :], in_=rden[:, :nt, :])
                o = epool.tile([P, T_EP, HC], F32, tag="o")
                nc.vector.tensor_tensor(
                    out=o[:, :nt, :].rearrange("p t (h c) -> p t h c", c=C),
                    in0=num.rearrange("p t (h c) -> p t h c", c=C),
                    in1=rden[:, :nt, :].unsqueeze(3)
                        .to_broadcast([P, nt, H, C]),
                    op=OP.mult)
                nc.vector.tensor_tensor(
                    out=o[:, :nt, :], in0=o[:, :nt, :],
                    in1=bias_b[:].unsqueeze(1).to_broadcast([P, nt, HC]),
                    op=OP.add)
                xr = epool.tile([P, T_EP, HC], F32, tag="xr")
                nc.sync.dma_start(
                    out=xr[:, :nt, :],
                    in_=xres_d[t0 * P:(t0 + nt) * P, :]
                        .rearrange("(t p) c -> p t c", p=P))
                nc.vector.tensor_tensor(out=o[:, :nt, :], in0=o[:, :nt, :],
                                        in1=xr[:, :nt, :], op=OP.add)
                # LayerNorm across channels
                mu = epool.tile([P, T_EP], F32, tag="mu")
                nc.vector.reduce_sum(out=mu[:, :nt], in_=o[:, :nt, :],
                                     axis=AX.X)
                nc.scalar.mul(out=mu[:, :nt], in_=mu[:, :nt], mul=1.0 / HC)
                nc.vector.tensor_tensor(
                    out=o[:, :nt, :], in0=o[:, :nt, :],
                    in1=mu[:, :nt].unsqueeze(2).to_broadcast([P, nt, HC]),
                    op=OP.subtract)
                sq = epool.tile([P, T_EP, HC], F32, tag="sq")
                nc.vector.tensor_tensor(out=sq[:, :nt, :], in0=o[:, :nt, :],
                                        in1=o[:, :nt, :], op=OP.mult)
                var = epool.tile([P, T_EP], F32, tag="var")
                nc.vector.reduce_sum(out=var[:, :nt], in_=sq[:, :nt, :],
                                     axis=AX.X)
                # rstd = exp(-0.5 * ln(var/HC + eps))
                nc.scalar.activation(out=var[:, :nt], in_=var[:, :nt],
                                     func=AF.Ln, scale=1.0 / HC,
                                     bias=eps_t[:, 0:1])
                nc.scalar.activation(out=var[:, :nt], in_=var[:, :nt],
                                     func=AF.Exp, scale=-0.5)
                nc.vector.tensor_tensor(
                    out=o[:, :nt, :], in0=o[:, :nt, :],
                    in1=var[:, :nt].unsqueeze(2).to_broadcast([P, nt, HC]),
                    op=OP.mult)
                nc.vector.tensor_tensor(
                    out=o[:, :nt, :], in0=o[:, :nt, :],
                    in1=gamma_b[:].unsqueeze(1).to_broadcast([P, nt, HC]),
                    op=OP.mult)
                nc.vector.tensor_tensor(
                    out=o[:, :nt, :], in0=o[:, :nt, :],
                    in1=beta_b[:].unsqueeze(1).to_broadcast([P, nt, HC]),
                    op=OP.add)
                # ELU = relu(x) + min(exp(x)-1, 0)
                ex = epool.tile([P, T_EP, HC], F32, tag="ex")
                nc.scalar.activation(out=ex[:, :nt, :], in_=o[:, :nt, :],
                                     func=AF.Exp)
                nc.vector.tensor_scalar(out=ex[:, :nt, :], in0=ex[:, :nt, :],
                                        scalar1=-1.0, scalar2=0.0,
                                        op0=OP.add, op1=OP.min)
                nc.scalar.activation(out=o[:, :nt, :], in_=o[:, :nt, :],
                                     func=AF.Relu)
                nc.vector.tensor_tensor(out=o[:, :nt, :], in0=o[:, :nt, :],
                                        in1=ex[:, :nt, :], op=OP.add)
                nc.sync.dma_start(
                    out=out_d[t0 * P:(t0 + nt) * P, :]
                        .rearrange("(t p) c -> p t c", p=P),
                    in_=o[:, :nt, :])

            if debug_stage < 4:
                zout = cpool.tile([P, TILES, HC], F32, tag="zout")
                nc.gpsimd.memset(zout[:], 0.0)
                nc.sync.dma_start(
                    out=out_d[:].rearrange("(t p) c -> p t c", p=P),
                    in_=zout[:])

            qrr = [0]  # SWDGE queue round-robin across all gather calls
            for (tb, s_t) in supertiles():
                if debug_stage < 2:
                    break
                nch = s_t * M                   # chunks this supertile
                cb = tb * M                     # global chunk base
                n_lo = s_t * M_LO
                n_hi = s_t * M_HI

                idx_lo = gpool.tile([P, S_SUP * M_LO * 8], I16, tag="idxlo")
                nc.sync.dma_start(
                    out=idx_lo[:, :n_lo * 8],
                    in_=idxlo_d[:, tb * M_LO * 8:(tb * M_LO + n_lo) * 8])
                idx_hi = gpool.tile([P, S_SUP * M_HI * 8], I16, tag="idxhi")
                nc.sync.dma_start(
                    out=idx_hi[:, :n_hi * 8],
                    in_=idxhi_d[:, tb * M_HI * 8:(tb * M_HI + n_hi) * 8])

                g = gpool.tile([P, S_SUP * M, ROW], U8, tag="g")
                GB = 8  # chunks per gather (1024 idx limit)
                for c0 in range(0, n_lo, GB):
                    k = min(GB, n_lo - c0)
                    nc.gpsimd.dma_gather(
                        g[:, c0:c0 + k, :], tbl[0:SPLIT, :],
                        idx_lo[:, c0 * 8:(c0 + k) * 8],
                        k * P, k * P, ROW, queue_num=qrr[0] % 4)
                    qrr[0] += 1
                for c0 in range(0, n_hi, GB):
                    k = min(GB, n_hi - c0)
                    nc.gpsimd.dma_gather(
                        g[:, n_lo + c0:n_lo + c0 + k, :], tbl[SPLIT:, :],
                        idx_hi[:, c0 * 8:(c0 + k) * 8],
                        k * P, k * P, ROW, queue_num=qrr[0] % 4)
                    qrr[0] += 1

                if debug_stage < 3:
                    continue
                oh_sb = spool.tile([P, S_SUP * M, P], U8, tag="oh")
                nc.sync.dma_start(
                    out=oh_sb[:, :nch, :],
                    in_=oh_d[:, cb:cb + nch, :])
                ohdt_sb = spool.tile([P, S_SUP * M, P], U8, tag="ohdt")
                nc.sync.dma_start(
                    out=ohdt_sb[:, :nch, :],
                    in_=ohdt_d[:, cb:cb + nch, :])
                ea_sb = spool.tile([P, S_SUP * M, ED], BF16, tag="ea")
                nc.sync.dma_start(
                    out=ea_sb[:, :nch, :],
                    in_=ea_d[:, cb:cb + nch, :])

                # chunk -> (tile, is_first, is_last) in supertile slot order
                owner = ([tb + p for p in range(s_t) for _ in range(M_LO)]
                         + [tb + p for p in range(s_t) for _ in range(M_HI)])
                first = {tb + p: p * M_LO for p in range(s_t)}
                last = {tb + p: n_lo + p * M_HI + M_HI - 1
                        for p in range(s_t)}

                # sdst = OHdt^T s_own   (per chunk, 4 cols)
                al_ps = papool.tile([P, S_SUP * M * H], F32, tag="al")
                for cidx in range(nch):
                    t = owner[cidx]
                    sl = slice(cidx * H, (cidx + 1) * H)
                    nc.tensor.matmul(
                        out=al_ps[:, sl],
                        lhsT=ohdt_sb[:, cidx, :].bitcast(FP8),
                        rhs=s_own[:, t * H:(t + 1) * H],
                        start=True, stop=True, skip_group_check=True)

                # ae = (ea * C16t) tree-summed over ED, on DVE (bf16)
                prod = wpool.tile([P, S_SUP * M, H, ED], BF16, tag="prod")
                nc.vector.tensor_tensor(
                    out=prod[:, :nch],
                    in0=ea_sb[:, :nch, :].unsqueeze(2)
                        .to_broadcast([P, nch, H, ED]),
                    in1=c16b[:].unsqueeze(1).to_broadcast([P, nch, H, ED]),
                    op=OP.mult)
                for w in (8, 4, 2, 1):
                    nc.vector.tensor_tensor(
                        out=prod[:, :nch, :, 0:w],
                        in0=prod[:, :nch, :, 0:w],
                        in1=prod[:, :nch, :, w:2 * w],
                        op=OP.add)

                # alpha += aj (gathered);  lrelu;  exp -> msg tail
                alpha = wpool.tile([P, S_SUP * M, H], F32, tag="alpha")
                nc.vector.tensor_tensor(
                    out=alpha[:, :nch, :],
                    in0=al_ps[:, :nch * H].rearrange("p (c h) -> p c h", h=H),
                    in1=g[:, :nch, HC:HC + 2 * H].bitcast(BF16),
                    op=OP.add)
                nc.vector.tensor_tensor(
                    out=alpha[:, :nch, :],
                    in0=alpha[:, :nch, :],
                    in1=prod[:, :nch, :, 0],
                    op=OP.add)
                nc.vector.scalar_tensor_tensor(
                    out=alpha[:, :nch, :], in0=alpha[:, :nch, :],
                    scalar=NEG_SLOPE, in1=alpha[:, :nch, :],
                    op0=OP.mult, op1=OP.max)
                msg = wpool.tile([P, S_SUP * M, HC + H], BF16, tag="msg")
                nc.scalar.activation(out=msg[:, :nch, HC:HC + H],
                                     in_=alpha[:, :nch, :], func=AF.Exp)
                # msg = e * xl
                nc.vector.tensor_tensor(
                    out=msg[:, :nch, 0:HC].rearrange(
                        "p c (h w) -> p c h w", w=C),
                    in0=g[:, :nch, 0:HC].bitcast(FP8).rearrange(
                        "p c (h w) -> p c h w", w=C),
                    in1=msg[:, :nch, HC:HC + H].unsqueeze(3)
                        .to_broadcast([P, nch, H, C]),
                    op=OP.mult)

                if debug_stage < 4:
                    continue
                # accumulate [numerator | denom] per tile
                accs = {}
                for cidx in range(nch):
                    t = owner[cidx]
                    if cidx == first[t]:
                        accs[t] = pcpool.tile([P, HC + H], F32,
                                              name="acc_t", tag="acc")
                    nc.tensor.matmul(
                        out=accs[t][:],
                        lhsT=oh_sb[:, cidx, :].bitcast(FP8),
                        rhs=msg[:, cidx, :],
                        start=(cidx == first[t]), stop=(cidx == last[t]))

                for p in range(s_t):
                    t = tb + p
                    if ep_fill == 0:
                        stage_ep = epool.tile([P, T_EP, HC + H], F32,
                                              tag="stage_ep")
                        ep_base = t
                    nc.vector.tensor_copy(out=stage_ep[:, t - ep_base, :],
                                          in_=accs[t][:])
                    ep_fill += 1
                    if ep_fill == T_EP or t == TILES - 1:
                        if debug_stage >= 5:
                            flush_epilogue(stage_ep, ep_fill, ep_base)
                        else:
                            nc.sync.dma_start(
                                out=out_d[ep_base * P:(ep_base + ep_fill) * P, :]
                                    .rearrange("(t p) c -> p t c", p=P),
                                in_=stage_ep[:, :ep_fill, 0:HC])
                        ep_fill = 0

    nc.compile()
    return nc


# --------------------------------------------------------------------------
# entry point
# --------------------------------------------------------------------------

def kernel(**inputs) -> np.ndarray:
    in_maps, M_LO, M_HI = host_prep(**inputs)
    nc = build_program(M_LO, M_HI)
    res = run_bass_kernel_spmd(nc, in_maps, list(range(N_CORES)))
    parts = [res.results[c]["out"][:NPC] for c in range(N_CORES)]
    return np.concatenate(parts, axis=0).astype(np.float32)



# revision 12
# speedup vs baseline: 3.6964x; 1.6448x over previous
"""MultiHeadGAT layer on 8 Trainium2 NeuronCores — v3 (gather-free streams).

Strategy (graph/data parallel, dst-sharded, per sharding hint):
  - Nodes partitioned into 8 ranges (6250/core); each core owns its output
    rows.  Edges routed host-side to the core/tile owning their destination,
    padded to 128-edge chunks.  All params + x replicated; no collectives.
  - No node table and no dma_gather: the host streams x^T with columns
    repeated in edge-slot order (a pure permutation/replication of the
    input), so every DMA is a big contiguous HWDGE transfer.  Per chunk one
    matmul  lhsT=x_jT[128k,128lane] @ rhsBT[128k, 132]  produces
    xl_j = W_lin x_j (128 cols) and aj = xl_j . att_dst (4 cols) in PSUM.
  - alpha = lrelu(aj + OHdt^T s_own + (ea*C16) tree-sum);  e = exp(alpha)
    (no segment-max shift; alphas bounded);  msg = e * xl_j with channels
    stored (c,h)-interleaved so the DVE runs its 2x bf16 mode (e is read
    along the innermost stride-1 head axis).  One-hot matmuls accumulate
    [numerator | denom] per tile in PSUM; the epilogue's first multiply
    un-interleaves back to standard (h,c) channel order for free.
  - Epilogue batched 8 tiles wide: divide, +bias, +residual, LayerNorm
    (rstd via exp(-0.5 ln(var+eps))), ELU.
"""

import math

import numpy as np

import concourse.bass as bass
import concourse.bacc as bacc
import concourse.mybir as mybir
from concourse.tile import TileContext
from concourse.masks import make_identity
from concourse.bass_utils import run_bass_kernel_spmd

F32 = mybir.dt.float32
BF16 = mybir.dt.bfloat16
FP8 = mybir.dt.float8e4
U8 = mybir.dt.uint8
AF = mybir.ActivationFunctionType
OP = mybir.AluOpType
AX = mybir.AxisListType

H, C = 4, 32
HC = H * C          # 128
IN_CH = 128
ED = 16
NEG_SLOPE = 0.2
LN_EPS = 1e-5
P = 128
T_EP = 8            # tiles per epilogue batch
G3 = 3              # chunks per ps_x psum bank (3*132 <= 512)

N_NODES = 50000
N_CORES = 8
NPC = N_NODES // N_CORES          # 6250
TILES = math.ceil(NPC / P)        # 49
NPAD = TILES * P                  # 6272

ONE_FP8 = 0x38  # 1.0 in float8_e4m3


# --------------------------------------------------------------------------
# host-side routing (index bookkeeping + layout only)
# --------------------------------------------------------------------------

def host_prep(x, edge_index, edge_attr, W_lin, W_edge, att_src, att_dst,
              att_edge, bias, ln_gamma, ln_beta):
    import ml_dtypes
    bf = ml_dtypes.bfloat16

    src = np.asarray(edge_index[0], np.int64)
    dst = np.asarray(edge_index[1], np.int64)
    ea = np.asarray(edge_attr, np.float32)
    E = src.shape[0]

    core_of = dst // NPC
    local = dst - core_of * NPC
    tile_of = local >> 7
    rel = local & 127

    key = core_of * TILES + tile_of
    order = np.argsort(key, kind="stable")
    key_s = key[order]
    counts = np.bincount(key_s, minlength=N_CORES * TILES)
    M = max(1, int(math.ceil(counts.max() / P)))
    C_TOT = TILES * M

    group_start = np.zeros(N_CORES * TILES, np.int64)
    np.cumsum(counts[:-1], out=group_start[1:])
    rank = np.arange(E, dtype=np.int64) - group_start[key_s]

    t_s = tile_of[order]
    c_s = core_of[order]
    rel_s = rel[order]
    src_s = src[order]
    ea_s = ea[order]

    chunk = t_s * M + (rank >> 7)
    lane = rank & 127

    x = np.asarray(x, np.float32)
    xTbf = np.ascontiguousarray(x.T).astype(bf)   # [128, N]

    wl = np.asarray(W_lin, np.float32).astype(bf)
    we = np.asarray(W_edge, np.float32).astype(bf)
    a_src = np.asarray(att_src, np.float32).reshape(HC, 1).astype(bf)
    a_dst = np.asarray(att_dst, np.float32).reshape(HC, 1).astype(bf)
    a_edge = np.asarray(att_edge, np.float32).reshape(HC, 1).astype(bf)
    bias_r = np.asarray(bias, np.float32).reshape(1, HC)
    gamma_r = np.asarray(ln_gamma, np.float32).reshape(1, HC)
    beta_r = np.asarray(ln_beta, np.float32).reshape(1, HC)

    in_maps = []
    for c in range(N_CORES):
        m = c_s == c
        ch = chunk[m]
        ln = lane[m]
        rl = rel_s[m]
        sc = src_s[m]
        slot = ch * P + ln

        # x^T replicated into edge-slot order (pad slots -> 0)
        xjT = np.zeros((IN_CH, C_TOT * P), bf)
        xjT[:, slot] = xTbf[:, sc]

        # partition-major one-hots: [P, C_TOT, P]
        oh = np.zeros((C_TOT, P, P), np.uint8)
        oh[ch, ln, rl] = ONE_FP8
        oh_pm = np.ascontiguousarray(oh.transpose(1, 0, 2))
        ohdt = np.ascontiguousarray(oh.transpose(2, 0, 1))

        eat = np.zeros((C_TOT, P, ED), np.float32)
        eat[ch, ln] = ea_s[m]
        eat = np.ascontiguousarray(eat.transpose(1, 0, 2)).astype(bf)

        n0 = c * NPC
        xres = np.zeros((NPAD, IN_CH), np.float32)
        xres[:NPC] = x[n0:n0 + NPC]
        xresT = np.ascontiguousarray(xres.T).astype(bf)

        in_maps.append(dict(
            xjT=xjT,
            xresT=xresT,
            xres=xres,
            oh=oh_pm,
            ohdt=ohdt,
            ea_sw=eat,
            wl=wl,
            we=we,
            a_src=a_src,
            a_dst=a_dst,
            a_edge=a_edge,
            bias=bias_r,
            ln_gamma=gamma_r,
            ln_beta=beta_r,
        ))
    return in_maps, M


# --------------------------------------------------------------------------
# device program
# --------------------------------------------------------------------------

def build_program(M, num_devices=None):
    C_TOT = TILES * M

    nc = bacc.Bacc("TRN2", target_bir_lowering=False, debug=False,
                   num_devices=num_devices or N_CORES)

    dp = nc.declare_dram_parameter
    xjT_d = dp("xjT", [IN_CH, C_TOT * P], BF16, isOutput=False)
    xresT_d = dp("xresT", [IN_CH, NPAD], BF16, isOutput=False)
    xres_d = dp("xres", [NPAD, IN_CH], F32, isOutput=False)
    oh_d = dp("oh", [P, C_TOT, P], U8, isOutput=False)
    ohdt_d = dp("ohdt", [P, C_TOT, P], U8, isOutput=False)
    ea_d = dp("ea_sw", [P, C_TOT, ED], BF16, isOutput=False)
    wl_d = dp("wl", [HC, IN_CH], BF16, isOutput=False)
    we_d = dp("we", [HC, ED], BF16, isOutput=False)
    asrc_d = dp("a_src", [HC, 1], BF16, isOutput=False)
    adst_d = dp("a_dst", [HC, 1], BF16, isOutput=False)
    aedge_d = dp("a_edge", [HC, 1], BF16, isOutput=False)
    bias_d = dp("bias", [1, HC], F32, isOutput=False)
    gamma_d = dp("ln_gamma", [1, HC], F32, isOutput=False)
    beta_d = dp("ln_beta", [1, HC], F32, isOutput=False)
    out_d = dp("out", [NPAD, HC], F32, isOutput=True)

    with TileContext(nc) as tc:
        with (
            tc.tile_pool(name="const", bufs=1) as cpool,
            tc.tile_pool(name="stream", bufs=2) as spool,
            tc.tile_pool(name="work", bufs=2) as wpool,
            tc.tile_pool(name="ep", bufs=2) as epool,
            tc.tile_pool(name="ps_a", bufs=1, space="PSUM") as papool,
            tc.tile_pool(name="ps_x", bufs=3, space="PSUM") as pxpool,
            tc.tile_pool(name="ps_al", bufs=2, space="PSUM") as plpool,
            tc.tile_pool(name="ps_acc", bufs=2, space="PSUM") as pcpool,
        ):
            # ---------------- phase A: constants --------------------------
            ident = cpool.tile([P, P], BF16, tag="ident")
            make_identity(nc, ident[:])

            wl_sb = cpool.tile([HC, IN_CH], BF16, tag="wl")
            nc.sync.dma_start(out=wl_sb[:], in_=wl_d[:])
            we_sb = cpool.tile([HC, ED], BF16, tag="we")
            nc.sync.dma_start(out=we_sb[:], in_=we_d[:])
            asrc = cpool.tile([HC, 1], BF16, tag="asrc")
            nc.sync.dma_start(out=asrc[:], in_=asrc_d[:])
            adst = cpool.tile([HC, 1], BF16, tag="adst")
            nc.sync.dma_start(out=adst[:], in_=adst_d[:])
            aedge = cpool.tile([HC, 1], BF16, tag="aedge")
            nc.sync.dma_start(out=aedge[:], in_=aedge_d[:])

            # block-diagonal attention matrices [HC, 3H]:
            # cols 0:H att_dst (aj, src side), H:2H att_src (s_own, dst
            # side), 2H:3H att_edge.
            a_bd = cpool.tile([HC, 3 * H], BF16, tag="a_bd")
            nc.gpsimd.memset(a_bd[:], 0.0)
            for h in range(H):
                sl = slice(h * C, (h + 1) * C)
                nc.vector.tensor_copy(out=a_bd[sl, h:h + 1], in_=adst[sl, :])
                nc.vector.tensor_copy(out=a_bd[sl, H + h:H + h + 1],
                                      in_=asrc[sl, :])
                nc.vector.tensor_copy(out=a_bd[sl, 2 * H + h:2 * H + h + 1],
                                      in_=aedge[sl, :])

            # rhsBT [in_ch, 136] = [ W_lin^T | B_dst(aj) | B_src(s_own) ]
            rhsBT = cpool.tile([IN_CH, HC + 2 * H], BF16, tag="rhsbt")
            wlT_ps = papool.tile([P, P], BF16, tag="psA")
            nc.tensor.transpose(out=wlT_ps[:], in_=wl_sb[:], identity=ident[:])
            nc.scalar.copy(out=rhsBT[:, 0:HC], in_=wlT_ps[:])
            b8_ps = papool.tile([IN_CH, 2 * H], F32, tag="psA")
            nc.tensor.matmul(out=b8_ps[:], lhsT=wl_sb[:],
                             rhs=a_bd[:, 0:2 * H], start=True, stop=True)
            nc.vector.tensor_copy(out=rhsBT[:, HC:HC + 2 * H], in_=b8_ps[:])

            c16_ps = papool.tile([ED, H], F32, tag="psA")
            nc.tensor.matmul(out=c16_ps[:], lhsT=we_sb[:],
                             rhs=a_bd[:, 2 * H:3 * H], start=True, stop=True)
            c16 = cpool.tile([ED, H], BF16, tag="c16")
            nc.vector.tensor_copy(out=c16[:], in_=c16_ps[:])
            # c16T [H, ED] -> broadcast [P, H, ED] for the DVE ae product
            c16t_ps = papool.tile([H, ED], BF16, tag="psA")
            nc.tensor.transpose(out=c16t_ps[:], in_=c16[:],
                                identity=ident[0:ED, 0:ED])
            c16t = cpool.tile([H, ED], BF16, tag="c16t")
            nc.vector.tensor_copy(out=c16t[:], in_=c16t_ps[:])
            c16t_dram = nc.dram_tensor("c16t_scratch", [H, ED], BF16)
            nc.sync.dma_start(out=c16t_dram[:], in_=c16t[:])
            c16b = cpool.tile([P, H, ED], BF16, tag="c16b")
            nc.sync.dma_start(
                out=c16b[:],
                in_=c16t_dram[:].rearrange("a b -> (a b)")
                    .unsqueeze(0).to_broadcast([P, H * ED]))

            bias_b = cpool.tile([P, HC], F32, tag="bias_b")
            nc.sync.dma_start(out=bias_b[:], in_=bias_d[:].to_broadcast([P, HC]))
            gamma_b = cpool.tile([P, HC], F32, tag="gamma_b")
            nc.sync.dma_start(out=gamma_b[:],
                              in_=gamma_d[:].to_broadcast([P, HC]))
            beta_b = cpool.tile([P, HC], F32, tag="beta_b")
            nc.sync.dma_start(out=beta_b[:], in_=beta_d[:].to_broadcast([P, HC]))

            eps_t = cpool.tile([P, 1], F32, tag="eps_t")
            nc.gpsimd.memset(eps_t[:], LN_EPS)
            tiny_t = cpool.tile([P, 1], F32, tag="tiny_t")
            nc.gpsimd.memset(tiny_t[:], 1e-16)

            # s_own [128, TILES*H] bf16 (xl . att_src for own nodes)
            xresT_sb = cpool.tile([IN_CH, NPAD], BF16, tag="xresT")
            nc.sync.dma_start(out=xresT_sb[:], in_=xresT_d[:])
            s_own = cpool.tile([P, TILES * H], BF16, tag="s_own")
            for t in range(TILES):
                so_ps = papool.tile([P, H], F32, tag="psA")
                nc.tensor.matmul(out=so_ps[:],
                                 lhsT=xresT_sb[:, t * P:(t + 1) * P],
                                 rhs=rhsBT[:, HC + H:HC + 2 * H],
                                 start=True, stop=True)
                nc.vector.tensor_copy(out=s_own[:, t * H:(t + 1) * H],
                                      in_=so_ps[:])

            # ---------------- phase C: edges (per dst tile) ---------------
            stage_ep = None
            ep_fill = 0
            ep_base = 0

            def flush_epilogue(stage_ep, n_tiles, t0):
                # stage_ep: [P, T_EP, HC+H] f32, tiles t0..t0+n_tiles-1.
                # num cols 0:HC are (c,h)-interleaved; the first multiply
                # below restores standard (h,c) order via a strided read.
                nt = n_tiles
                num = stage_ep[:, :nt, 0:HC]
                den = stage_ep[:, :nt, HC:HC + H]
                rden = epool.tile([P, T_EP, H], F32, tag="rden")
                nc.scalar.activation(out=rden[:, :nt, :], in_=den,
                                     func=AF.Identity, bias=tiny_t[:, 0:1])
                nc.vector.reciprocal(out=rden[:, :nt, :], in_=rden[:, :nt, :])
                o = epool.tile([P, T_EP, HC], F32, tag="o")
                nc.vector.tensor_tensor(
                    out=o[:, :nt, :].rearrange("p t (h c) -> p t h c", c=C),
                    in0=num.rearrange("p t (c h) -> p t h c", h=H),
                    in1=rden[:, :nt, :].unsqueeze(3)
                        .to_broadcast([P, nt, H, C]),
                    op=OP.mult)
                nc.vector.tensor_tensor(
                    out=o[:, :nt, :], in0=o[:, :nt, :],
                    in1=bias_b[:].unsqueeze(1).to_broadcast([P, nt, HC]),
                    op=OP.add)
                xr = epool.tile([P, T_EP, HC], F32, tag="xr")
                nc.sync.dma_start(
                    out=xr[:, :nt, :],
                    in_=xres_d[t0 * P:(t0 + nt) * P, :]
                        .rearrange("(t p) c -> p t c", p=P))
                nc.vector.tensor_tensor(out=o[:, :nt, :], in0=o[:, :nt, :],
                                        in1=xr[:, :nt, :], op=OP.add)
                # LayerNorm across channels
                mu = epool.tile([P, T_EP], F32, tag="mu")
                nc.vector.reduce_sum(out=mu[:, :nt], in_=o[:, :nt, :],
                                     axis=AX.X)
                nc.scalar.mul(out=mu[:, :nt], in_=mu[:, :nt], mul=1.0 / HC)
                nc.vector.tensor_tensor(
                    out=o[:, :nt, :], in0=o[:, :nt, :],
                    in1=mu[:, :nt].unsqueeze(2).to_broadcast([P, nt, HC]),
                    op=OP.subtract)
                sq = epool.tile([P, T_EP, HC], F32, tag="sq")
                nc.vector.tensor_tensor(out=sq[:, :nt, :], in0=o[:, :nt, :],
                                        in1=o[:, :nt, :], op=OP.mult)
                var = epool.tile([P, T_EP], F32, tag="var")
                nc.vector.reduce_sum(out=var[:, :nt], in_=sq[:, :nt, :],
                                     axis=AX.X)
                # rstd = exp(-0.5 * ln(var/HC + eps))
                nc.scalar.activation(out=var[:, :nt], in_=var[:, :nt],
                                     func=AF.Ln, scale=1.0 / HC,
                                     bias=eps_t[:, 0:1])
                nc.scalar.activation(out=var[:, :nt], in_=var[:, :nt],
                                     func=AF.Exp, scale=-0.5)
                nc.vector.tensor_tensor(
                    out=o[:, :nt, :], in0=o[:, :nt, :],
                    in1=var[:, :nt].unsqueeze(2).to_broadcast([P, nt, HC]),
                    op=OP.mult)
                nc.vector.tensor_tensor(
                    out=o[:, :nt, :], in0=o[:, :nt, :],
                    in1=gamma_b[:].unsqueeze(1).to_broadcast([P, nt, HC]),
                    op=OP.mult)
                nc.vector.tensor_tensor(
                    out=o[:, :nt, :], in0=o[:, :nt, :],
                    in1=beta_b[:].unsqueeze(1).to_broadcast([P, nt, HC]),
                    op=OP.add)
                # ELU = relu(x) + min(exp(x)-1, 0)
                ex = epool.tile([P, T_EP, HC], F32, tag="ex")
                nc.scalar.activation(out=ex[:, :nt, :], in_=o[:, :nt, :],
                                     func=AF.Exp)
                nc.vector.tensor_scalar(out=ex[:, :nt, :], in0=ex[:, :nt, :],
                                        scalar1=-1.0, scalar2=0.0,
                                        op0=OP.add, op1=OP.min)
                nc.scalar.activation(out=o[:, :nt, :], in_=o[:, :nt, :],
                                     func=AF.Relu)
                nc.vector.tensor_tensor(out=o[:, :nt, :], in0=o[:, :nt, :],
                                        in1=ex[:, :nt, :], op=OP.add)
                nc.sync.dma_start(
                    out=out_d[t0 * P:(t0 + nt) * P, :]
                        .rearrange("(t p) c -> p t c", p=P),
                    in_=o[:, :nt, :])

            for t in range(TILES):
                cb = t * M

                xjT_sb = spool.tile([IN_CH, M * P], BF16, tag="xjt")
                nc.sync.dma_start(out=xjT_sb[:],
                                  in_=xjT_d[:, cb * P:(cb + M) * P])
                oh_sb = spool.tile([P, M, P], U8, tag="oh")
                nc.sync.dma_start(out=oh_sb[:], in_=oh_d[:, cb:cb + M, :])
                ohdt_sb = spool.tile([P, M, P], U8, tag="ohdt")
                nc.sync.dma_start(out=ohdt_sb[:], in_=ohdt_d[:, cb:cb + M, :])
                ea_sb = spool.tile([P, M, ED], BF16, tag="ea")
                nc.sync.dma_start(out=ea_sb[:], in_=ea_d[:, cb:cb + M, :])

                # xl_j | aj per chunk via PE;  xj_sb gets xl (c,h)-interleaved
                xj_sb = wpool.tile([P, M, HC], BF16, tag="xj")
                alpha = wpool.tile([P, M, H], F32, tag="alpha")
                for g0 in range(0, M, G3):
                    k = min(G3, M - g0)
                    psx = pxpool.tile([P, G3, HC + H], F32, tag="psx")
                    for i in range(k):
                        nc.tensor.matmul(
                            out=psx[:, i, :],
                            lhsT=xjT_sb[:, (g0 + i) * P:(g0 + i + 1) * P],
                            rhs=rhsBT[:, 0:HC + H],
                            start=True, stop=True)
                    nc.scalar.copy(
                        out=xj_sb[:, g0:g0 + k, :]
                            .rearrange("p m (c h) -> p m c h", h=H),
                        in_=psx[:, :k, 0:HC]
                            .rearrange("p m (h c) -> p m c h", c=C))
                    nc.scalar.copy(out=alpha[:, g0:g0 + k, :],
                                   in_=psx[:, :k, HC:HC + H])

                # sdst = OHdt^T s_own  (per chunk, 4 cols)
                al_ps = plpool.tile([P, M * H], F32, tag="al")
                for cidx in range(M):
                    sl = slice(cidx * H, (cidx + 1) * H)
                    nc.tensor.matmul(
                        out=al_ps[:, sl],
                        lhsT=ohdt_sb[:, cidx, :].bitcast(FP8),
                        rhs=s_own[:, t * H:(t + 1) * H],
                        start=True, stop=True, skip_group_check=True)

                # ae = (ea * C16t) tree-summed over ED, on DVE (bf16)
                prod = wpool.tile([P, M, H, ED], BF16, tag="prod")
                nc.vector.tensor_tensor(
                    out=prod[:],
                    in0=ea_sb[:].unsqueeze(2).to_broadcast([P, M, H, ED]),
                    in1=c16b[:].unsqueeze(1).to_broadcast([P, M, H, ED]),
                    op=OP.mult)
                for w in (8, 4, 2, 1):
                    nc.vector.tensor_tensor(
                        out=prod[:, :, :, 0:w],
                        in0=prod[:, :, :, 0:w],
                        in1=prod[:, :, :, w:2 * w],
                        op=OP.add)

                # alpha = lrelu(aj + sdst + ae)
                nc.vector.tensor_tensor(
                    out=alpha[:],
                    in0=alpha[:],
                    in1=al_ps[:].rearrange("p (m h) -> p m h", h=H),
                    op=OP.add)
                nc.vector.tensor_tensor(
                    out=alpha[:], in0=alpha[:], in1=prod[:, :, :, 0],
                    op=OP.add)
                nc.vector.scalar_tensor_tensor(
                    out=alpha[:], in0=alpha[:],
                    scalar=NEG_SLOPE, in1=alpha[:],
                    op0=OP.mult, op1=OP.max)

                # msg = e * xl  ((c,h)-interleaved, DVE 2x bf16 mode)
                msg = wpool.tile([P, M, HC + H], BF16, tag="msg")
                nc.scalar.activation(out=msg[:, :, HC:HC + H],
                                     in_=alpha[:], func=AF.Exp)
                nc.vector.tensor_tensor(
                    out=msg[:, :, 0:HC].rearrange("p m (c h) -> p m c h", h=H),
                    in0=xj_sb[:].rearrange("p m (c h) -> p m c h", h=H),
                    in1=msg[:, :, HC:HC + H].unsqueeze(2)
                        .to_broadcast([P, M, C, H]),
                    op=OP.mult)

                # accumulate [numerator | denom] for this tile
                acc = pcpool.tile([P, HC + H], F32, name="acc_t", tag="acc")
                for cidx in range(M):
                    nc.tensor.matmul(
                        out=acc[:],
                        lhsT=oh_sb[:, cidx, :].bitcast(FP8),
                        rhs=msg[:, cidx, :],
                        start=(cidx == 0), stop=(cidx == M - 1))

                if ep_fill == 0:
                    stage_ep = epool.tile([P, T_EP, HC + H], F32,
                                          tag="stage_ep")
                    ep_base = t
                nc.vector.tensor_copy(out=stage_ep[:, t - ep_base, :],
                                      in_=acc[:])
                ep_fill += 1
                if ep_fill == T_EP or t == TILES - 1:
                    flush_epilogue(stage_ep, ep_fill, ep_base)
                    ep_fill = 0

    nc.compile()
    return nc


# --------------------------------------------------------------------------
# entry point
# --------------------------------------------------------------------------

def kernel(**inputs) -> np.ndarray:
    in_maps, M = host_prep(**inputs)
    nc = build_program(M)
    res = run_bass_kernel_spmd(nc, in_maps, list(range(N_CORES)))
    parts = [res.results[c]["out"][:NPC] for c in range(N_CORES)]
    return np.concatenate(parts, axis=0).astype(np.float32)
